# revision 1
# baseline (speedup 1.0000x reference)
"""AAM + Control-Contrastive loss on 8 TRN2 NeuronCores.

Sharding:
  - AAM classifier branch: classes C=10000 sharded 1250/core (weight arrives
    pre-transposed [D, Cs] bf16); x replicated as xT [D, B] bf16.
  - Contrastive BxB block: column-sharded 256/core; weight_m/weight_n arrive
    pre-gathered by label, pre-normalized, pre-transposed [D, 256] bf16.
  - One ReduceScatter feeds the ap_m-dependent phi_nm sweep on device; all
    per-row partial sums are emitted per-core and combined on the host
    (the unshard step), so no AllReduce sits on the critical tail.

Per core: phase 1 computes the contrastive q*k block (one 512-wide bf16
matmul per (b-tile, k-chunk)) plus its masked row sums, then fires the
ReduceScatter. Phase 2 sweeps the 1250 classes with fused exp+accumulate
epilogues while the RS + phi_nm sweeps overlap. The one-hot/phi label
correction is computed from gathered weight[label] rows and folded into
the partials.
"""

import math

import numpy as np

B = 2048
D = 512
C = 10000
NCORES = 8
CS = C // NCORES          # 1250 classes per core
JS = B // NCORES          # 256 contrastive columns per core
NB = B // 128             # 16 batch tiles
KD = D // 128             # 4 contraction chunks

# wide rhs layout: classes [0:1250) | wsum 1250:1252 | pad | wm [1280:1536) | wk [1536:1792)
W_QM = 1280
W_QK = 1536
W_ALL = 1792
NCB = CS - 1024           # classes in the third segment = 226

M_ = 0.2
S_ = 30.0
COS_M = math.cos(M_)
SIN_M = math.sin(M_)
TAN_M = SIN_M / COS_M
TH = math.cos(math.pi - M_)
MM = math.sin(math.pi - M_) * M_
EPS_LS = 0.1
EXP_SHIFT = -30.0
MASK_NEG = -1000.0

_CACHE = {}


def _build():
    import concourse.bacc as bacc
    import concourse.mybir as mybir
    import concourse.tile as tile

    f32 = mybir.dt.float32
    bf16 = mybir.dt.bfloat16
    op = mybir.AluOpType
    act = mybir.ActivationFunctionType
    X = mybir.AxisListType.X

    nc = bacc.Bacc("TRN2", target_bir_lowering=False, debug=False,
                   num_devices=NCORES)

    # ---- kernel I/O ----
    xT = nc.dram_tensor("xT", [D, B], bf16, kind="ExternalInput")
    wT = nc.dram_tensor("wT", [D, CS], bf16, kind="ExternalInput")
    wmT = nc.dram_tensor("wmT", [D, JS], bf16, kind="ExternalInput")
    wkT = nc.dram_tensor("wkT", [D, JS], bf16, kind="ExternalInput")
    mT = nc.dram_tensor("mT", [D, JS], bf16, kind="ExternalInput")
    xjT = nc.dram_tensor("xjT", [D, JS], bf16, kind="ExternalInput")
    lmd = nc.dram_tensor("lmd", [B, JS], bf16, kind="ExternalInput")
    dmd = nc.dram_tensor("dmd", [B, JS], bf16, kind="ExternalInput")
    invcnt_own = nc.dram_tensor("invcnt_own", [JS], f32, kind="ExternalInput")
    sel = nc.dram_tensor("sel", [1, 32], f32, kind="ExternalInput")
    out_d = nc.dram_tensor("out", [1, 128], f32, kind="ExternalOutput")
    outv_d = nc.dram_tensor("outv", [5, B], f32, kind="ExternalOutput")

    rg = [list(range(NCORES))]

    with tile.TileContext(nc) as tc:
        with (
            tc.tile_pool(name="pers", bufs=1) as pers,
            tc.tile_pool(name="sq", bufs=2) as sqp,
            tc.tile_pool(name="mask", bufs=2) as maskp,
            tc.tile_pool(name="psA", bufs=2, space="PSUM") as psA,   # [128,1024] x2
            tc.tile_pool(name="psD", bufs=2, space="PSUM") as psD,   # [128,256]  x2
            tc.tile_pool(name="psE", bufs=2, space="PSUM") as psE,   # [128,512]  x2
            tc.tile_pool(name="dram", bufs=1, space="DRAM") as dram,
        ):
            # ---------------- persistent SBUF tensors ----------------
            xts = [pers.tile([128, B], bf16, name=f"xt{k}", tag=f"xt{k}") for k in range(KD)]
            walls = [pers.tile([128, W_ALL], bf16, name=f"wall{k}", tag=f"wall{k}") for k in range(KD)]
            mts = [pers.tile([128, JS], bf16, name=f"mt{k}", tag=f"mt{k}") for k in range(KD)]
            xjs = [pers.tile([128, JS], bf16, name=f"xj{k}", tag=f"xj{k}") for k in range(KD)]
            sim = pers.tile([128, NB * JS], f32, name="sim", tag="sim")
            san = pers.tile([128, NB * JS], f32, name="san", tag="san")
            cbf = pers.tile([128, NB * JS], f32, name="cbf", tag="cbf")
            lmb16 = pers.tile([128, NB * JS], bf16, name="lmb16", tag="lmb16")
            dmb16 = pers.tile([128, NB * JS], bf16, name="dmb16", tag="dmb16")
            trash = pers.tile([128, 1280], f32, name="trash", tag="trash")

            ones_col = pers.tile([128, 1], bf16, name="ones_col", tag="ones_col")
            ones_row = pers.tile([1, 128], f32, name="ones_row", tag="ones_row")
            shift_col = pers.tile([128, 1], f32, name="shift_col", tag="shift_col")
            nc.vector.memset(ones_col[:, :], 1.0)
            nc.vector.memset(ones_row[:, :], 1.0)
            nc.vector.memset(shift_col[:, :], EXP_SHIFT)

            invcnt_ob = pers.tile([128, JS], f32, name="invcnt_ob", tag="invcnt_ob")
            nc.sync.dma_start(out=invcnt_ob[:, :],
                              in_=invcnt_own[None, :].broadcast_to((128, JS)))
            sel_sb = pers.tile([1, 32], f32, name="sel_sb", tag="sel_sb")
            nc.sync.dma_start(out=sel_sb[:, :], in_=sel[:, :])

            nc.gpsimd.dma_start(out=lmb16[:, :].rearrange("p (t j) -> p t j", j=JS),
                              in_=lmd[:, :].rearrange("(t p) j -> p t j", p=128))
            nc.gpsimd.dma_start(out=dmb16[:, :].rearrange("p (t j) -> p t j", j=JS),
                              in_=dmd[:, :].rearrange("(t p) j -> p t j", p=128))

            # ---------------- load big operands ----------------
            for k in range(KD):
                ksl = slice(k * 128, (k + 1) * 128)
                nc.sync.dma_start(out=walls[k][:, W_QM:W_QM + JS], in_=wmT[ksl, :])
                nc.sync.dma_start(out=walls[k][:, W_QK:W_QK + JS], in_=wkT[ksl, :])
                for g in range(4):
                    gsl = slice(g * 512, (g + 1) * 512)
                    nc.sync.dma_start(out=xts[k][:, gsl], in_=xT[ksl, gsl])
                for o, w in ((0, 512), (512, 512), (1024, 226)):
                    nc.gpsimd.dma_start(out=walls[k][:, o:o + w], in_=wT[ksl, o:o + w])
                nc.vector.memset(walls[k][:, CS + 2:W_QM], 0.0)
                nc.gpsimd.dma_start(out=mts[k][:, :], in_=mT[ksl, :])
                nc.gpsimd.dma_start(out=xjs[k][:, :], in_=xjT[ksl, :])

            # ---------------- x column norms -> invn (as [128,16]) ----------------
            invn_row = pers.tile([1, B], f32, name="invn_row", tag="invn_row")
            for g in range(4):
                ps = psE.tile([1, 512], f32, name="psrow", tag="E")
                for k in range(KD):
                    sq = sqp.tile([128, 512], bf16, name="sq", tag="sq")
                    nc.scalar.square(sq[:, :], xts[k][:, g * 512:(g + 1) * 512])
                    nc.tensor.matmul(ps[:, :], ones_col[:, :], sq[:, :],
                                     start=(k == 0), stop=(k == KD - 1))
                nc.vector.tensor_copy(invn_row[:, g * 512:(g + 1) * 512], ps[:, :])
            invn_dram = dram.tile([B], f32, name="invn_dram", tag="invn_dram")
            nc.sync.dma_start(out=invn_dram[:].unsqueeze(0), in_=invn_row[0:1, :])
            invn_c = pers.tile([128, NB], f32, name="invn_c", tag="invn_c")
            nc.sync.dma_start(out=invn_c[:, :], in_=invn_dram[:].rearrange("(t p) -> p t", p=128))
            nc.scalar.activation(invn_c[:, :], invn_c[:, :], act.Abs_reciprocal_sqrt)
            sinvn_c = pers.tile([128, NB], f32, name="sinvn_c", tag="sinvn_c")
            invn2_c = pers.tile([128, NB], f32, name="invn2_c", tag="invn2_c")
            nc.vector.tensor_single_scalar(sinvn_c[:, :], invn_c[:, :], S_, op.mult)
            nc.vector.tensor_tensor(invn2_c[:, :], invn_c[:, :], invn_c[:, :], op.mult)

            # ---------------- phase 1: contrastive q*k sweep ----------------
            rsA = pers.tile([128, 2, NB], f32, name="rsA", tag="rsA")    # [rssl, ap]
            rsB = pers.tile([128, 3, NB], f32, name="rsB", tag="rsB")    # [rs_out, rs_exp, labval]
            aexp = pers.tile([128, NB, 2], f32, name="aexp", tag="aexp")
            nc.vector.memset(rsA[:, :, :], 0.0)
            nc.vector.memset(rsB[:, :, :], 0.0)
            nc.vector.memset(aexp[:, :, :], 0.0)

            for t in range(NB):
                pe1 = psE.tile([128, 512], f32, name="pe1", tag="E")
                for k in range(KD):
                    lhs = xts[k][:, t * 128:(t + 1) * 128]
                    nc.tensor.matmul(pe1[:, :], lhs, walls[k][:, W_QM:W_ALL],
                                     start=(k == 0), stop=(k == KD - 1))
                blk = slice(t * JS, (t + 1) * JS)
                qs = maskp.tile([128, JS], f32, name="qs", tag="qs")
                nc.scalar.activation(qs[:, :], pe1[:, 0:JS], act.Copy)
                nc.vector.tensor_tensor(sim[:, blk], qs[:, :],
                                        pe1[:, JS:2 * JS], op.mult)
                nc.vector.scalar_tensor_tensor(
                    trash[:, :JS], sim[:, blk], 1.0, lmb16[:, blk],
                    op.mult, op.mult, accum_out=rsA[:, 0, t:t + 1])
                nc.vector.scalar_tensor_tensor(
                    trash[:, :JS], sim[:, blk], 1.0, dmb16[:, blk],
                    op.mult, op.mult, accum_out=rsA[:, 1, t:t + 1])

            # row sums were accumulated on raw q*k; apply invn^2 now (tiny)
            nc.vector.tensor_tensor(rsA[:, 0, :], rsA[:, 0, :], invn2_c[:, :], op.mult)
            nc.vector.tensor_tensor(rsA[:, 1, :], rsA[:, 1, :], invn2_c[:, :], op.mult)

            # ---------------- ReduceScatter of masked row sums ----------------
            arA_in = dram.tile([2, B], f32, name="arA_in", tag="arA_in")
            rs_sc = dram.tile([JS], f32, name="rs_sc", tag="rs_sc")
            nc.sync.dma_start(out=arA_in[:, :].rearrange("v (t p) -> p v t", p=128),
                              in_=rsA[:, :, :])
            nc.gpsimd.collective_compute(
                "ReduceScatter", op.add, ins=[arA_in[0, :].opt()],
                outs=[rs_sc[:].opt()], replica_groups=rg)
            # rsA partials also go straight to the host
            nc.sync.dma_start(out=outv_d[3:5, :].rearrange("v (t p) -> p v t", p=128),
                              in_=rsA[:, :, :])
            # scale the stored raw q*k block to true sim for the phi_nm sweep
            for t in range(NB):
                blk = slice(t * JS, (t + 1) * JS)
                nc.vector.tensor_scalar(sim[:, blk], sim[:, blk],
                                        invn2_c[:, t:t + 1], None, op.mult)

            # ---------------- wsum columns (classes arrive pre-normalized) -------
            wsum_tmp = pers.tile([128, 1], f32, name="wsum_tmp", tag="wsum_tmp")
            for k in range(KD):
                nc.scalar.activation(trash[:, :CS], walls[k][:, :CS],
                                     act.Copy, accum_out=wsum_tmp[:, :])
                nc.vector.tensor_copy(walls[k][:, CS:CS + 1], wsum_tmp[:, :])
                nc.vector.tensor_copy(walls[k][:, CS + 1:CS + 2], wsum_tmp[:, :])

            # ---------------- phase 2: class sweep ----------------
            for t in range(NB):
                pa = psA.tile([128, 1024], f32, name="pa", tag="A")
                pd = psD.tile([128, JS], f32, name="pd", tag="D")
                for k in range(KD):
                    lhs = xts[k][:, t * 128:(t + 1) * 128]
                    nc.tensor.matmul(pa[:, 0:512], lhs, walls[k][:, 0:512],
                                     start=(k == 0), stop=(k == KD - 1))
                    nc.tensor.matmul(pa[:, 512:1024], lhs, walls[k][:, 512:1024],
                                     start=(k == 0), stop=(k == KD - 1))
                    nc.tensor.matmul(pd[:, :], lhs, walls[k][:, 1024:1280],
                                     start=(k == 0), stop=(k == KD - 1))
                nc.scalar.activation(trash[:, :1024], pa[:, :], act.Exp,
                                     bias=shift_col[:, :], scale=sinvn_c[:, t:t + 1],
                                     accum_out=aexp[:, t, 0:1])
                nc.scalar.activation(trash[:, :NCB], pd[:, :NCB], act.Exp,
                                     bias=shift_col[:, :], scale=sinvn_c[:, t:t + 1],
                                     accum_out=aexp[:, t, 1:2])
                nc.scalar.activation(rsB[:, 0, t:t + 1], pd[:, NCB:NCB + 1],
                                     act.Copy, scale=sinvn_c[:, t:t + 1])

            # ---------------- label-column correction (phi at label) ----------------
            iota_pm = pers.tile([128, JS], f32, name="iota_pm", tag="iota_pm")
            nc.gpsimd.iota(iota_pm[:, :], pattern=[[1, JS]], base=0,
                           channel_multiplier=-1, allow_small_or_imprecise_dtypes=True)
            cosl = pers.tile([128, 2], f32, name="cosl", tag="cosl")
            for h in range(2):
                pc = psD.tile([128, JS], f32, name="pcosl", tag="D")
                for k in range(KD):
                    lhs = xjs[k][:, h * 128:(h + 1) * 128]
                    nc.tensor.matmul(pc[:, :], lhs, mts[k][:, :],
                                     start=(k == 0), stop=(k == KD - 1))
                msk = maskp.tile([128, JS], f32, name="msk", tag="msk")
                nc.vector.tensor_single_scalar(msk[:, :], iota_pm[:, :],
                                               float(128 * h), op.is_equal)
                nc.vector.scalar_tensor_tensor(
                    trash[:, :JS], pc[:, :], 1.0, msk[:, :],
                    op.mult, op.mult, accum_out=cosl[:, h:h + 1])
            invn_own = pers.tile([128, 2], f32, name="invn_own", tag="invn_own")
            selb = pers.tile([128, 2, 16], f32, name="selb", tag="selb")
            for h in range(2):
                bc2 = psE.tile([128, 512], f32, name="bcast", tag="E")
                nc.tensor.matmul(bc2[:, :16], ones_row[:, :],
                                 sel_sb[0:1, h * 16:(h + 1) * 16],
                                 start=True, stop=True)
                nc.vector.tensor_copy(selb[:, h, :], bc2[:, :16])
                nc.vector.scalar_tensor_tensor(
                    trash[:, :16], invn_c[:, :], 1.0, selb[:, h, :],
                    op.mult, op.mult, accum_out=invn_own[:, h:h + 1])
            nc.vector.tensor_tensor(cosl[:, :], cosl[:, :], invn_own[:, :], op.mult)

            c2 = pers.tile([128, 2], f32, name="c2", tag="c2")
            nc.vector.tensor_tensor(c2[:, :], cosl[:, :], cosl[:, :], op.mult)
            nc.vector.tensor_scalar(c2[:, :], c2[:, :], -1.0, 1.0, op.mult, op.add)
            nc.vector.tensor_scalar(c2[:, :], c2[:, :], 0.0, 1.0, op.max, op.min)
            nc.scalar.sqrt(c2[:, :], c2[:, :])
            p0 = pers.tile([128, 2], f32, name="p0", tag="p0")
            nc.vector.tensor_single_scalar(p0[:, :], cosl[:, :], COS_M, op.mult)
            nc.vector.scalar_tensor_tensor(p0[:, :], c2[:, :], -SIN_M, p0[:, :],
                                           op.mult, op.add)
            cond = pers.tile([128, 2], f32, name="cond", tag="cond")
            nc.vector.tensor_single_scalar(cond[:, :], cosl[:, :], TH, op.is_gt)
            alt = pers.tile([128, 2], f32, name="alt", tag="alt")
            nc.vector.tensor_single_scalar(alt[:, :], cosl[:, :], MM, op.subtract)
            nc.vector.tensor_tensor(p0[:, :], p0[:, :], alt[:, :], op.subtract)
            nc.vector.tensor_tensor(p0[:, :], cond[:, :], p0[:, :], op.mult)
            phil = pers.tile([128, 2], f32, name="phil", tag="phil")
            nc.vector.tensor_tensor(phil[:, :], alt[:, :], p0[:, :], op.add)

            labcorr = pers.tile([128, 2], f32, name="labcorr", tag="labcorr")
            outcorr = pers.tile([128, 2], f32, name="outcorr", tag="outcorr")
            expcorr = pers.tile([128, 2], f32, name="expcorr", tag="expcorr")
            e2t = pers.tile([128, 2], f32, name="e2t", tag="e2t")
            nc.vector.tensor_single_scalar(labcorr[:, :], phil[:, :], S_, op.mult)
            nc.vector.tensor_tensor(outcorr[:, :], phil[:, :], cosl[:, :], op.subtract)
            nc.vector.tensor_single_scalar(outcorr[:, :], outcorr[:, :], S_, op.mult)
            nc.scalar.activation(expcorr[:, :], phil[:, :], act.Exp,
                                 bias=shift_col[:, :], scale=S_)
            nc.scalar.activation(e2t[:, :], cosl[:, :], act.Exp,
                                 bias=shift_col[:, :], scale=S_)
            nc.vector.tensor_tensor(expcorr[:, :], expcorr[:, :], e2t[:, :], op.subtract)
            for h in range(2):
                nc.vector.scalar_tensor_tensor(rsB[:, 0, :], selb[:, h, :],
                                               outcorr[:, h:h + 1], rsB[:, 0, :],
                                               op.mult, op.add)
                nc.vector.scalar_tensor_tensor(rsB[:, 1, :], selb[:, h, :],
                                               expcorr[:, h:h + 1], rsB[:, 1, :],
                                               op.mult, op.add)
                nc.vector.scalar_tensor_tensor(rsB[:, 2, :], selb[:, h, :],
                                               labcorr[:, h:h + 1], rsB[:, 2, :],
                                               op.mult, op.add)

            tmp16 = pers.tile([128, NB], f32, name="tmp16", tag="tmp16")
            nc.vector.tensor_reduce(tmp16[:, :], aexp[:, :, :], X, op.add)
            nc.vector.tensor_tensor(rsB[:, 1, :], rsB[:, 1, :], tmp16[:, :], op.add)
            nc.sync.dma_start(out=outv_d[0:3, :].rearrange("v (t p) -> p v t", p=128),
                              in_=rsB[:, :, :])

            # ---------------- contrastive phi_nm sweep (needs RS) ----------------
            cosb = pers.tile([128, JS], f32, name="cosb", tag="cosb")
            sinb = pers.tile([128, JS], f32, name="sinb", tag="sinb")
            nc.sync.dma_start(out=cosb[:, :],
                              in_=rs_sc[None, :].broadcast_to((128, JS)))
            nc.vector.tensor_tensor(cosb[:, :], cosb[:, :], invcnt_ob[:, :], op.mult)
            nc.vector.tensor_scalar(cosb[:, :], cosb[:, :], 0.0, 1.0, op.max, op.min)
            nc.scalar.activation(sinb[:, :], cosb[:, :], act.Sqrt,
                                 bias=1.0, scale=-1.0)

            sen_cols = pers.tile([128, 4], f32, name="sen_cols", tag="sen_cols")
            cosb_b = cosb[:, :].unsqueeze(1).broadcast_to((128, 4, JS))
            sinb_b = sinb[:, :].unsqueeze(1).broadcast_to((128, 4, JS))
            for cchunk in range(4):
                ck = slice(cchunk * 1024, (cchunk + 1) * 1024)
                sim_k = sim[:, ck]
                san_k = san[:, ck]
                cbf_k = cbf[:, ck]
                nc.vector.tensor_scalar(sim_k, sim_k, 0.0, 1.0, op.max, op.min)
                nc.scalar.activation(san_k, sim_k, act.Sqrt, bias=1.0, scale=-1.0)
                nc.vector.tensor_tensor(
                    san_k.rearrange("p (t j) -> p t j", j=JS), 
                    san_k.rearrange("p (t j) -> p t j", j=JS), cosb_b, op.mult)
                nc.vector.tensor_tensor(
                    sim_k.rearrange("p (t j) -> p t j", j=JS),
                    sim_k.rearrange("p (t j) -> p t j", j=JS), sinb_b, op.mult)
                nc.vector.tensor_tensor(san_k, san_k, sim_k, op.add)
                nc.scalar.activation(cbf_k, san_k, act.Relu, bias=1.0, scale=-1.0)
                nc.scalar.sqrt(cbf_k, cbf_k)
                nc.vector.scalar_tensor_tensor(cbf_k, cbf_k, -TAN_M, san_k,
                                               op.mult, op.add)
                nc.vector.scalar_tensor_tensor(cbf_k, lmb16[:, ck], MASK_NEG, cbf_k,
                                               op.mult, op.add)
                nc.scalar.activation(san_k, cbf_k, act.Exp, scale=COS_M,
                                     accum_out=sen_cols[:, cchunk:cchunk + 1])
            sen_col = pers.tile([128, 1], f32, name="sen_col", tag="sen_col")
            nc.vector.tensor_reduce(sen_col[:, :], sen_cols[:, :], X, op.add)
            nc.sync.dma_start(out=out_d[:, :].rearrange("o b -> b o"),
                              in_=sen_col[:, :])

    nc.compile()
    return nc


def _prep_inputs(x, label, weight, weight_m, weight_n):
    import ml_dtypes
    bf = ml_dtypes.bfloat16
    lab = np.asarray(label).astype(np.int64)
    x = np.asarray(x, dtype=np.float32)
    weight = np.asarray(weight, dtype=np.float32)
    weight_m = np.asarray(weight_m, dtype=np.float32)
    weight_n = np.asarray(weight_n, dtype=np.float32)

    cnt = np.bincount(lab, minlength=C).astype(np.float32)
    invcnt = (1.0 / cnt[lab]).astype(np.float32)
    xT = np.ascontiguousarray(x.T).astype(bf)

    def nrm(a):
        return a / np.maximum(np.linalg.norm(a, axis=1, keepdims=True), 1e-12)

    in_maps = []
    for i in range(NCORES):
        js = slice(i * JS, (i + 1) * JS)
        labj = lab[js]
        lmm = (lab[:, None] == labj[None, :]).astype(bf)
        dmm = np.zeros((B, JS), dtype=bf)
        dmm[np.arange(i * JS, (i + 1) * JS), np.arange(JS)] = 1
        sel = np.zeros((1, 32), dtype=np.float32)
        sel[0, 2 * i] = 1.0
        sel[0, 16 + 2 * i + 1] = 1.0
        in_maps.append({
            "xT": xT,
            "wT": np.ascontiguousarray(nrm(weight[i * CS:(i + 1) * CS]).T).astype(bf),
            "wmT": np.ascontiguousarray(nrm(weight_m[labj]).T).astype(bf),
            "wkT": np.ascontiguousarray(nrm(weight_n[labj]).T).astype(bf),
            "mT": np.ascontiguousarray(nrm(weight[labj]).T).astype(bf),
            "xjT": np.ascontiguousarray(xT[:, js]),
            "lmd": lmm,
            "dmd": dmm,
            "invcnt_own": np.ascontiguousarray(invcnt[js]),
            "sel": sel,
        })
    return in_maps


def kernel(**inputs):
    from concourse.bass_utils import run_bass_kernel_spmd

    if "nc" not in _CACHE:
        _CACHE["nc"] = _build()
    nc = _CACHE["nc"]

    lab = np.asarray(inputs["label"]).astype(np.int64)
    in_maps = _prep_inputs(**inputs)
    res = run_bass_kernel_spmd(nc, in_maps, core_ids=list(range(NCORES)))

    # host-side unshard/combine (float64)
    sen = sum(float(np.sum(r["out"])) for r in res.results)
    pv = np.zeros((5, B), dtype=np.float64)
    for r in res.results:
        pv += r["outv"].astype(np.float64)
    rs_out, rs_exp, labval, rssl, ap = pv

    aam_terms = (1.0 - EPS_LS) * labval + (EPS_LS / C) * rs_out \
        - (30.0 + np.log(rs_exp))
    aam_loss = -np.mean(aam_terms)

    cnt = np.bincount(lab, minlength=C).astype(np.float64)[lab]
    ap_m = np.clip(rssl / cnt, 0.0, 1.0)
    cos_ap = np.clip(ap, 0.0, 1.0)
    sin_ap = np.sqrt(np.clip(1.0 - cos_ap, 0.0, 1.0))
    sin_apm = np.sqrt(np.clip(1.0 - ap_m, 0.0, 1.0))
    pc = cos_ap * ap_m - sin_ap * sin_apm
    ps = np.sqrt(np.clip(1.0 - pc, 0.0, 1.0))
    phi_pm = pc * COS_M - ps * SIN_M
    s_neg = float(np.sum(np.exp(1.0 - phi_pm)))

    z = math.log(sen) + math.log(s_neg)
    cc_loss = np.logaddexp(0.0, z)
    return np.array(aam_loss + cc_loss, dtype=np.float32)



# revision 4
# speedup vs baseline: 3.0830x; 3.0830x over previous
"""AAM + Control-Contrastive loss on 8 TRN2 NeuronCores (no collectives).

Key identity: sim[r,c] depends on c only through label[c], so the masked
row-mean ap_m equals diagonal(sim) exactly.  Each core computes its own
256 diag values locally from elementwise xj*wm / xj*wk reductions -- the
ReduceScatter of the baseline is gone.

Sharding:
  - AAM classifier: classes C=10000 sharded 1250/core.  x normalized on
    host, weights normalized+scaled on host, both cast fp8e4 (x16 scale);
    class sweep runs DoubleRow fp8 matmuls (2 k-chunks per instr).
  - Contrastive BxB block column-sharded 256/core, also fp8 DoubleRow.
  - Label-column corrections (phi at label) are computed on HOST from a
    [1,256] cosl row each core exports; same for phi_pm/s_neg from the
    exported diag row.

All inputs arrive as two packed DRAM tensors mirroring the SBUF layout
(128 partitions x cols) so each load is a handful of large-descriptor
DMAs.  Outputs are one [128,33] f32 tile + one [2,256] f32 row pair.

Scalar engine runs a single Exp era (class sweep) then a single Sqrt era
(contrastive sweep) then Exp again -- 3 activation table loads total.
"""

import math

import numpy as np

B = 2048
D = 512
C = 10000
NCORES = 8
CS = C // NCORES          # 1250 classes per core
JS = B // NCORES          # 256 contrastive columns per core
NB = B // 128             # 16 batch tiles
KD = D // 128             # 4 contraction chunks
PR = 2                    # fp8 DoubleRow pairs (2 k-chunks each)

# packed fp8 tensor column offsets
X8O = 0                   # [2 pair][2 i][2048 b]
W8O = X8O + PR * 2 * B    # 8192: [2 pair][2 i][1280 c]
WCOLS = 1280              # 1250 classes | 1250: wsum | pad
MKO = W8O + PR * 2 * WCOLS        # 13312: [2 pair][2 i][512 j]  (wm|wk)
LMO = MKO + PR * 2 * 512          # 15360: [16 t][256 j] mask
F8 = LMO + NB * JS                # 19456

# packed bf16 tensor column offsets: xj | wm | wk | mt, each [4 k][256 j]
B16 = 4 * KD * JS                 # 4096

FP8_SCALE = 16.0
MM_SCALE = FP8_SCALE * FP8_SCALE  # matmul output scale (256)

M_ = 0.2
S_ = 30.0
COS_M = math.cos(M_)
SIN_M = math.sin(M_)
TAN_M = SIN_M / COS_M
TH = math.cos(math.pi - M_)
MM = math.sin(math.pi - M_) * M_
EPS_LS = 0.1
EXP_SHIFT = -30.0
MASK_NEG = -1000.0

_CACHE = {}


def _build():
    import concourse.bacc as bacc
    import concourse.mybir as mybir
    import concourse.tile as tile

    f32 = mybir.dt.float32
    bf16 = mybir.dt.bfloat16
    f8 = mybir.dt.float8e4
    op = mybir.AluOpType
    act = mybir.ActivationFunctionType
    DR = mybir.MatmulPerfMode.DoubleRow

    nc = bacc.Bacc("TRN2", target_bir_lowering=False, debug=False,
                   num_devices=NCORES)

    pk8_d = nc.dram_tensor("pk8", [128, F8], f8, kind="ExternalInput")
    pk16_d = nc.dram_tensor("pk16", [128, B16], bf16, kind="ExternalInput")
    outA_d = nc.dram_tensor("outA", [128, 33], f32, kind="ExternalOutput")
    outB_d = nc.dram_tensor("outB", [2, JS], f32, kind="ExternalOutput")

    with tile.TileContext(nc) as tc:
        with (
            tc.tile_pool(name="pers", bufs=1) as pers,
            tc.tile_pool(name="qsp", bufs=2) as qsp,
            tc.tile_pool(name="prodp", bufs=4) as prodp,
            tc.tile_pool(name="psA", bufs=2, space="PSUM") as psA,   # [128,1024]
            tc.tile_pool(name="psD", bufs=2, space="PSUM") as psD,   # [128,256]
            tc.tile_pool(name="psE", bufs=2, space="PSUM") as psE,   # [128,512]
        ):
            pk8 = pers.tile([128, F8], f8, name="pk8", tag="pk8")
            pk16 = pers.tile([128, B16], bf16, name="pk16", tag="pk16")
            sim = pers.tile([128, NB * JS], bf16, name="sim", tag="sim")
            cbuf = pers.tile([128, NB * JS], bf16, name="cbuf", tag="cbuf")
            sbuf = pers.tile([128, NB * JS], bf16, name="sbuf", tag="sbuf")
            cosb = pers.tile([128, JS], bf16, name="cosb", tag="cosb")
            sinb = pers.tile([128, JS], bf16, name="sinb", tag="sinb")
            ones_col = pers.tile([128, 1], bf16, name="ones_col", tag="ones_col")
            ones_row = pers.tile([1, 128], bf16, name="ones_row", tag="ones_row")
            qrow = pers.tile([1, JS], bf16, name="qrow", tag="qrow")
            apq_row = pers.tile([1, JS], f32, name="apq_row", tag="apq_row")
            apq_bf = pers.tile([1, JS], bf16, name="apq_bf", tag="apq_bf")
            cosl_row = pers.tile([1, JS], f32, name="cosl_row", tag="cosl_row")
            aexp = pers.tile([128, NB, 2], f32, name="aexp", tag="aexp")
            sen2 = pers.tile([128, 2], f32, name="sen2", tag="sen2")
            outA = pers.tile([128, 33], f32, name="outA", tag="outA")

            shift_col = pers.tile([128, 1], f32, name="shift_col",
                                  tag="shift_col")
            nc.vector.memset(shift_col[:, :], EXP_SHIFT)
            nc.vector.memset(ones_col[:, :], 1.0)
            nc.vector.memset(ones_row[:, :], 1.0)
            nc.vector.memset(aexp[:, :, :], 0.0)
            nc.vector.memset(sen2[:, :], 0.0)

            # ---------------- loads (large-descriptor, split across queues) ----
            nc.sync.dma_start(out=pk8[:, 0:4096], in_=pk8_d[:, 0:4096])
            nc.gpsimd.dma_start(out=pk8[:, 4096:8192], in_=pk8_d[:, 4096:8192])
            nc.sync.dma_start(out=pk8[:, W8O:W8O + 2560],
                              in_=pk8_d[:, W8O:W8O + 2560])
            nc.gpsimd.dma_start(out=pk8[:, W8O + 2560:MKO],
                                in_=pk8_d[:, W8O + 2560:MKO])
            nc.sync.dma_start(out=pk8[:, MKO:LMO], in_=pk8_d[:, MKO:LMO])
            nc.gpsimd.dma_start(out=pk16[:, 0:2048], in_=pk16_d[:, 0:2048])
            nc.sync.dma_start(out=pk16[:, 2048:B16], in_=pk16_d[:, 2048:B16])
            nc.gpsimd.dma_start(out=pk8[:, LMO:F8], in_=pk8_d[:, LMO:F8])

            x8v = pk8[:, X8O:W8O].rearrange("p (r i b) -> p r i b", r=2, i=2)
            w8v = pk8[:, W8O:MKO].rearrange("p (r i c) -> p r i c", r=2, i=2)
            mk8v = pk8[:, MKO:LMO].rearrange("p (r i j) -> p r i j", r=2, i=2)
            lm8v = pk8[:, LMO:F8]
            xjv = pk16[:, 0:1024].rearrange("p (k j) -> p k j", k=KD)
            wmv = pk16[:, 1024:2048].rearrange("p (k j) -> p k j", k=KD)
            wkv = pk16[:, 2048:3072].rearrange("p (k j) -> p k j", k=KD)
            mtv = pk16[:, 3072:4096].rearrange("p (k j) -> p k j", k=KD)

            # ---------------- gpsimd: diag/cosl elementwise products ----------
            prods_qm = []
            prods_qk = []
            prods_mt = []
            for k in range(KD):
                pq = prodp.tile([128, JS], bf16, name=f"pq{k}", tag="pq")
                nc.gpsimd.tensor_tensor(pq[:, :], xjv[:, k, :], wmv[:, k, :],
                                        op.mult)
                prods_qm.append(pq)
            for k in range(KD):
                pq = prodp.tile([128, JS], bf16, name=f"pk{k}", tag="pq")
                nc.gpsimd.tensor_tensor(pq[:, :], xjv[:, k, :], wkv[:, k, :],
                                        op.mult)
                prods_qk.append(pq)
            for k in range(KD):
                pq = prodp.tile([128, JS], bf16, name=f"pm{k}", tag="pq")
                nc.gpsimd.tensor_tensor(pq[:, :], xjv[:, k, :], mtv[:, k, :],
                                        op.mult)
                prods_mt.append(pq)

            # ---------------- phase 2: AAM class sweep (fp8 DoubleRow) --------
            for t in range(NB):
                ts = slice(t * 128, (t + 1) * 128)
                pa = psA.tile([128, 1024], f32, name="pa", tag="A")
                pd = psD.tile([128, 256], f32, name="pd", tag="D")
                for pr in range(PR):
                    st = pr == 0
                    sp = pr == PR - 1
                    lhs = x8v[:, pr, :, ts]
                    nc.tensor.matmul(pa[:, 0:512], lhs, w8v[:, pr, :, 0:512],
                                     start=st, stop=sp, perf_mode=DR)
                    nc.tensor.matmul(pa[:, 512:1024], lhs,
                                     w8v[:, pr, :, 512:1024],
                                     start=st, stop=sp, perf_mode=DR)
                    nc.tensor.matmul(pd[:, :], lhs, w8v[:, pr, :, 1024:1280],
                                     start=st, stop=sp, perf_mode=DR)
                nc.scalar.activation(sbuf[:, 0:1024], pa[:, :], act.Exp,
                                     bias=shift_col[:, :], scale=S_ / MM_SCALE,
                                     accum_out=aexp[:, t, 0:1])
                nc.scalar.activation(sbuf[:, 1024:1250], pd[:, 0:226], act.Exp,
                                     bias=shift_col[:, :], scale=S_ / MM_SCALE,
                                     accum_out=aexp[:, t, 1:2])
                nc.vector.tensor_single_scalar(outA[:, t:t + 1],
                                               pd[:, 226:227],
                                               S_ / MM_SCALE, op.mult)

            # ---------------- diag reduce matmuls + broadcast -----------------
            qd = psD.tile([128, 256], f32, name="qd", tag="D")
            for k in range(KD):
                nc.tensor.matmul(qd[0:1, :], ones_col[:, :], prods_qm[k][:, :],
                                 start=(k == 0), stop=(k == KD - 1))
            kd = psD.tile([128, 256], f32, name="kd", tag="D")
            for k in range(KD):
                nc.tensor.matmul(kd[0:1, :], ones_col[:, :], prods_qk[k][:, :],
                                 start=(k == 0), stop=(k == KD - 1))
            # vector: rows (one PSUM operand max per instr)
            nc.vector.tensor_copy(qrow[:, :], qd[0:1, :])
            nc.vector.tensor_tensor(apq_row[:, :], qrow[:, :], kd[0:1, :],
                                    op.mult)
            nc.vector.tensor_scalar(apq_bf[:, :], apq_row[:, :], 0.0, 1.0,
                                    op.max, op.min)
            bc = psD.tile([128, 256], f32, name="bc", tag="D")
            nc.tensor.matmul(bc[:, :], ones_row[:, :], apq_bf[:, :],
                             start=True, stop=True)
            nc.vector.tensor_copy(cosb[:, :], bc[:, :])

            # ---------------- phase 1: contrastive q*k (fp8 DoubleRow) --------
            for t in range(NB):
                ts = slice(t * 128, (t + 1) * 128)
                pe = psE.tile([128, 512], f32, name="pe", tag="E")
                for pr in range(PR):
                    nc.tensor.matmul(pe[:, :], x8v[:, pr, :, ts],
                                     mk8v[:, pr, :, :],
                                     start=(pr == 0), stop=(pr == PR - 1),
                                     perf_mode=DR)
                qs = qsp.tile([128, JS], bf16, name="qs", tag="qs")
                nc.vector.tensor_copy(qs[:, :], pe[:, 0:256])
                nc.vector.scalar_tensor_tensor(
                    sim[:, t * JS:(t + 1) * JS], qs[:, :], 1.0 / (MM_SCALE * MM_SCALE),
                    pe[:, 256:512], op.mult, op.mult)

            # cosl reduce (tensor) -> row out (vector, emitted at tail)
            cosl_ps = psD.tile([128, 256], f32, name="cosl_ps", tag="D")
            for k in range(KD):
                nc.tensor.matmul(cosl_ps[0:1, :], ones_col[:, :],
                                 prods_mt[k][:, :],
                                 start=(k == 0), stop=(k == KD - 1))

            # ---------------- contrastive sweep (2 chunks of 2048) ------------
            HB = NB // 2 * JS  # 2048
            ch = [slice(0, HB), slice(HB, 2 * HB)]

            def bview(tile_, c):
                return tile_[:, :].unsqueeze(1).broadcast_to((128, NB // 2, JS))

            # V1: c = clip(sim, 0, 1)
            nc.vector.tensor_scalar(cbuf[:, ch[0]], sim[:, ch[0]], 0.0, 1.0,
                                    op.max, op.min)
            nc.vector.tensor_scalar(cbuf[:, ch[1]], sim[:, ch[1]], 0.0, 1.0,
                                    op.max, op.min)
            # scalar Sqrt era: sinb, S1 (s = sqrt(1-c)), later S2
            nc.scalar.activation(sinb[:, :], cosb[:, :], act.Sqrt,
                                 bias=1.0, scale=-1.0)
            nc.scalar.activation(sbuf[:, ch[0]], cbuf[:, ch[0]], act.Sqrt,
                                 bias=1.0, scale=-1.0)
            nc.scalar.activation(sbuf[:, ch[1]], cbuf[:, ch[1]], act.Sqrt,
                                 bias=1.0, scale=-1.0)
            for c in range(2):
                cc = ch[c]
                cr = cbuf[:, cc].rearrange("p (t j) -> p t j", j=JS)
                sr = sbuf[:, cc].rearrange("p (t j) -> p t j", j=JS)
                mr = sim[:, cc].rearrange("p (t j) -> p t j", j=JS)
                # V2: c *= sinb ; V3: s *= cosb ; V4: san = c + s (into sim)
                nc.vector.tensor_tensor(cr, cr, bview(sinb, c), op.mult)
                nc.vector.tensor_tensor(sr, sr, bview(cosb, c), op.mult)
                nc.vector.tensor_tensor(sim[:, cc], cbuf[:, cc], sbuf[:, cc],
                                        op.add)
                # V5: u = min(san, 1) (into cbuf)
                nc.vector.tensor_single_scalar(cbuf[:, cc], sim[:, cc], 1.0,
                                               op.min)
                # S2: v = sqrt(1-u) (into sbuf)
                nc.scalar.activation(sbuf[:, cc], cbuf[:, cc], act.Sqrt,
                                     bias=1.0, scale=-1.0)
                # V6: sim = san - TAN_M*v ; V7: sim += MASK_NEG*lm
                nc.vector.scalar_tensor_tensor(sim[:, cc], sbuf[:, cc], -TAN_M,
                                               sim[:, cc], op.mult, op.add)
                nc.vector.scalar_tensor_tensor(sim[:, cc], lm8v[:, cc],
                                               MASK_NEG, sim[:, cc],
                                               op.mult, op.add)
                # S3: exp(COS_M * phi') with accum
                nc.scalar.activation(sbuf[:, cc], sim[:, cc], act.Exp,
                                     scale=COS_M, accum_out=sen2[:, c:c + 1])

            # ---------------- outputs ----------------
            nc.vector.tensor_tensor(outA[:, 16:32], aexp[:, :, 0],
                                    aexp[:, :, 1], op.add)
            nc.vector.tensor_tensor(outA[:, 32:33], sen2[:, 0:1], sen2[:, 1:2],
                                    op.add)
            nc.vector.tensor_copy(cosl_row[:, :], cosl_ps[0:1, :])
            nc.sync.dma_start(out=outB_d[0:1, :], in_=apq_row[:, :])
            nc.sync.dma_start(out=outB_d[1:2, :], in_=cosl_row[:, :])
            nc.sync.dma_start(out=outA_d[:, :], in_=outA[:, :])

    nc.compile()
    return nc


def _prep_inputs(x, label, weight, weight_m, weight_n):
    import ml_dtypes
    bf = ml_dtypes.bfloat16
    f8 = ml_dtypes.float8_e4m3
    lab = np.asarray(label).astype(np.int64)
    x = np.asarray(x, dtype=np.float32)
    weight = np.asarray(weight, dtype=np.float32)
    weight_m = np.asarray(weight_m, dtype=np.float32)
    weight_n = np.asarray(weight_n, dtype=np.float32)

    def nrm(a):
        return a / np.maximum(np.linalg.norm(a, axis=1, keepdims=True), 1e-12)

    xn = nrm(x)
    xnT = np.ascontiguousarray(xn.T)                      # [512, 2048]

    def pack_cols(a):
        # [512, N] -> [128, 4*N] in (pair, i, col) SBUF layout
        n = a.shape[1]
        return a.reshape(2, 2, 128, n).transpose(2, 0, 1, 3).reshape(128, 4 * n)

    def pack_k(a):
        # [512, 256] -> [128, 1024] in (k, j) layout
        return a.reshape(4, 128, -1).transpose(1, 0, 2).reshape(128, -1)

    xr = pack_cols(FP8_SCALE * xnT)                       # [128, 8192]

    in_maps = []
    for i in range(NCORES):
        js = slice(i * JS, (i + 1) * JS)
        labj = lab[js]
        wn = nrm(weight[i * CS:(i + 1) * CS])             # [1250, 512]
        wcols = np.zeros((D, WCOLS), dtype=np.float32)
        wcols[:, 0:CS] = FP8_SCALE * wn.T
        wcols[:, CS] = FP8_SCALE * wn.sum(axis=0)
        wmn = nrm(weight_m[labj])
        wkn = nrm(weight_n[labj])
        mtn = nrm(weight[labj])
        mk = np.concatenate([FP8_SCALE * wmn.T, FP8_SCALE * wkn.T], axis=1)
        lm = (lab[:, None] == labj[None, :]).astype(np.float32)
        lmr = lm.reshape(NB, 128, JS).transpose(1, 0, 2).reshape(128, NB * JS)
        pk8 = np.concatenate(
            [xr, pack_cols(wcols), pack_cols(mk), lmr], axis=1).astype(f8)
        pk16 = np.concatenate(
            [pack_k(xnT[:, js]), pack_k(wmn.T), pack_k(wkn.T), pack_k(mtn.T)],
            axis=1).astype(bf)
        in_maps.append({"pk8": pk8, "pk16": pk16})
    return in_maps


def kernel(**inputs):
    from concourse.bass_utils import run_bass_kernel_spmd

    if "nc" not in _CACHE:
        _CACHE["nc"] = _build()
    nc = _CACHE["nc"]

    in_maps = _prep_inputs(**inputs)
    res = run_bass_kernel_spmd(nc, in_maps, core_ids=list(range(NCORES)))

    # ---------------- host-side combine (float64) ----------------
    rs_out = np.zeros(B)
    rs_exp = np.zeros(B)
    sen = 0.0
    ap = np.zeros(B)
    cosl = np.zeros(B)
    for i, r in enumerate(res.results):
        a = r["outA"].astype(np.float64)
        rs_out += a[:, 0:16].T.reshape(B)
        rs_exp += a[:, 16:32].T.reshape(B)
        sen += float(a[:, 32].sum())
        b = r["outB"].astype(np.float64)
        ap[i * JS:(i + 1) * JS] = b[0]
        cosl[i * JS:(i + 1) * JS] = b[1]

    # AAM: label-column corrections (phi at label)
    sine = np.sqrt(np.clip(1.0 - cosl * cosl, 0.0, 1.0))
    phi = np.where(cosl - TH > 0, cosl * COS_M - sine * SIN_M, cosl - MM)
    rs_out_full = rs_out + S_ * (phi - cosl)
    rs_exp_full = rs_exp + np.exp(S_ * phi - 30.0) - np.exp(S_ * cosl - 30.0)
    aam_terms = (1.0 - EPS_LS) * S_ * phi + (EPS_LS / C) * rs_out_full \
        - (30.0 + np.log(rs_exp_full))
    aam_loss = -np.mean(aam_terms)

    # Contrastive: ap_m == ap (diag identity)
    cos_ap = np.clip(ap, 0.0, 1.0)
    sin_ap = np.sqrt(np.clip(1.0 - cos_ap, 0.0, 1.0))
    pc = cos_ap * cos_ap - sin_ap * sin_ap
    ps = np.sqrt(np.clip(1.0 - pc, 0.0, 1.0))
    phi_pm = pc * COS_M - ps * SIN_M
    s_neg = float(np.sum(np.exp(1.0 - phi_pm)))

    z = math.log(sen) + math.log(s_neg)
    cc_loss = np.logaddexp(0.0, z)
    return np.array(aam_loss + cc_loss, dtype=np.float32)
